# revision 2
# baseline (speedup 1.0000x reference)
"""Trainium2 Bass kernel for the BiDirectionalRNN problem.

Math (matches the fp32 jax reference):
    e = emb[x]                                   # [B, T, 512]
    fwd:  h_t = relu(e_t @ Wf.T + bf + h_{t-1})  # fs[t]
    bwd over reversed e: bs[s]                   # generation order
    xcat = concat_t [fs[t], bs[t]]  -> [B, T*1024]
    h1 = relu(xcat @ W1.T + b1); 4x h = relu(h @ W2.T + b2); out = h @ Wo.T + bo

Strategy:
  * Data-parallel over batch: 1024/8 = 128 samples per NeuronCore.
  * Host folds the embedding + input projection: a_d[:, b, s] =
    (Wd @ emb.T)[:, x[b, s']] + bd  (s' reversed for bwd). The device never
    sees emb/Wf/Wb - only the per-step drive terms A, laid out for the DVE
    scan primitive.
  * The whole 32-step recurrence h = relu(a + h_prev) runs as ONE DVE
    tensor_tensor_scan per (dir, hid-tile): state=(a add state) max 0 along
    the free dim, fp32 internal state, with a -1e30 separator column between
    independent (hid, batch) chains.
  * W1 (64MB fp32) ships as bf16 (32MB/core), streamed in 64 [128,2048]
    tiles; the [B,32768]@[32768,512] GEMM accumulates all 256 k-chunks into
    a single PSUM bank, lhsT = strided views of the scan outputs.
  * Tail: PE-transpose h1, then 4 x [512,512] + [97,512] in transposed
    (feature-major) layout; biases via ACT per-partition bias / rank-1 matmul.
"""

import numpy as np
import ml_dtypes

BF16 = ml_dtypes.bfloat16

MOD = 97
HID = 512
T = 32
B = 1024
NCORES = 8
BL = B // NCORES          # 128 batch per core
CL = T + 1                # chain length incl. separator column
FREE = BL * CL            # 4224 scan columns per tile
NEG = -1e30
W1_GRP = 64               # W1 DMA groups of 4 k-chunks

_CACHE: dict = {}


def _build():
    import concourse.tile as tile
    from concourse import bacc, mybir

    fp32 = mybir.dt.float32
    bf16 = mybir.dt.bfloat16

    nc = bacc.Bacc(
        "TRN2", target_bir_lowering=False, debug=False, num_devices=NCORES
    )

    d = {
        "A": nc.dram_tensor("A", [8, 128, FREE], bf16, kind="ExternalInput").ap(),
        "W1S": nc.dram_tensor("W1S", [W1_GRP, 128, 2048], bf16, kind="ExternalInput").ap(),
        "W2S": nc.dram_tensor("W2S", [4, 128, 512], bf16, kind="ExternalInput").ap(),
        "WOS": nc.dram_tensor("WOS", [4, 128, MOD], bf16, kind="ExternalInput").ap(),
        "B1": nc.dram_tensor("B1", [1, 512], bf16, kind="ExternalInput").ap(),
        "B2T": nc.dram_tensor("B2T", [128, 4], fp32, kind="ExternalInput").ap(),
        "BO": nc.dram_tensor("BO", [128, 1], fp32, kind="ExternalInput").ap(),
        "IDN": nc.dram_tensor("IDN", [128, 128], bf16, kind="ExternalInput").ap(),
        "OUT": nc.dram_tensor("OUT", [MOD, BL], fp32, kind="ExternalOutput").ap(),
    }

    with tile.TileContext(nc) as tc:
        _emit(tc, d, mybir)

    nc.compile()
    return nc


def _emit(tc, d, mybir):
    nc = tc.nc
    fp32 = mybir.dt.float32
    bf16 = mybir.dt.bfloat16
    AF = mybir.ActivationFunctionType
    ALU = mybir.AluOpType

    from contextlib import ExitStack

    with ExitStack() as ctx:
        const = ctx.enter_context(tc.tile_pool(name="const", bufs=1))
        a_pool = ctx.enter_context(tc.tile_pool(name="apool", bufs=2))
        h_pool = ctx.enter_context(tc.tile_pool(name="hpool", bufs=1))
        w1_pool = ctx.enter_context(tc.tile_pool(name="w1pool", bufs=16))
        hp_pool = ctx.enter_context(tc.tile_pool(name="hppool", bufs=8))
        ps_h1 = ctx.enter_context(tc.tile_pool(name="psh1", bufs=1, space="PSUM"))
        ps_sm = ctx.enter_context(tc.tile_pool(name="pssm", bufs=2, space="PSUM"))

        # ---- constants ----
        w2sb = const.tile([128, 4 * 512], bf16)
        for k in range(4):
            nc.sync.dma_start(w2sb[:, k * 512:(k + 1) * 512], d["W2S"][k])
        wosb = const.tile([128, 4 * MOD], bf16)
        for k in range(4):
            nc.sync.dma_start(wosb[:, k * MOD:(k + 1) * MOD], d["WOS"][k])
        b1sb = const.tile([1, 512], bf16)
        nc.sync.dma_start(b1sb[:], d["B1"])
        b2sb = const.tile([128, 4], fp32)
        nc.sync.dma_start(b2sb[:], d["B2T"])
        bosb = const.tile([128, 1], fp32)
        nc.sync.dma_start(bosb[:], d["BO"])
        idsb = const.tile([128, 128], bf16)
        nc.sync.dma_start(idsb[:], d["IDN"])
        ones = const.tile([1, 128], bf16)
        nc.vector.memset(ones[:], 1.0)
        zero = const.tile([128, 1], bf16)
        nc.vector.memset(zero[:], 0.0)

        # ---- recurrence scans: h = relu(a + h_prev), j = dir*4 + hid_tile ----
        hs = []
        for j in range(8):
            a_t = a_pool.tile([128, FREE], bf16)
            nc.sync.dma_start(a_t[:], d["A"][j])
            h_t = h_pool.tile([128, FREE], bf16, tag=f"h{j}")
            nc.vector.tensor_tensor_scan(
                h_t[:], a_t[:], zero[:].broadcast_to([128, FREE]),
                initial=0.0, op0=ALU.add, op1=ALU.max,
            )
            hs.append(h_t)

        # ---- linear1: psum[b, n] = sum_k xcat.T[k-chunk].T @ W1.T[k-chunk] ----
        psum_h1 = ps_h1.tile([128, 512], fp32)
        for g in range(W1_GRP):
            w_t = w1_pool.tile([128, 2048], bf16)
            nc.sync.dma_start(w_t[:], d["W1S"][g])
            t_idx, dd = g // 2, g % 2
            for m in range(4):
                hv = hs[dd * 4 + m][:].rearrange("p (b t) -> p t b", t=CL)[:, t_idx, :]
                nc.tensor.matmul(
                    psum_h1[:], hv, w_t[:, m * 512:(m + 1) * 512],
                    start=(g == 0 and m == 0), stop=False,
                )
        # rank-1 bias: ones.T @ b1 broadcasts b1 over the batch partitions
        nc.tensor.matmul(psum_h1[:], ones[:], b1sb[:], start=False, stop=True)

        h1sb = const.tile([128, 512], bf16)
        nc.scalar.activation(h1sb[:], psum_h1[:], AF.Relu)

        # ---- transpose h1 to feature-major [512, 128] as 4 chunks ----
        cur = []
        for m in range(4):
            pt = ps_sm.tile([128, 128], bf16, tag="pt")
            nc.tensor.transpose(pt[:], h1sb[:, m * 128:(m + 1) * 128], idsb[:])
            hq = hp_pool.tile([128, 128], bf16, tag="hp")
            nc.scalar.copy(hq[:], pt[:])
            cur.append(hq)

        # ---- 4 x (h = relu(W2 @ h' + b2)) in feature-major form ----
        for _L in range(4):
            nxt = []
            for m in range(4):
                pl = ps_sm.tile([128, 128], fp32, tag="pl")
                for k in range(4):
                    nc.tensor.matmul(
                        pl[:],
                        w2sb[:, k * 512 + m * 128: k * 512 + m * 128 + 128],
                        cur[k][:],
                        start=(k == 0), stop=(k == 3),
                    )
                hq = hp_pool.tile([128, 128], bf16, tag="hp")
                nc.scalar.activation(hq[:], pl[:], AF.Relu, bias=b2sb[:, m:m + 1])
                nxt.append(hq)
            cur = nxt

        # ---- output head: out' = Wo @ h' + bo  -> [97, 128] ----
        po = ps_sm.tile([MOD, 128], fp32, tag="po")
        for k in range(4):
            nc.tensor.matmul(
                po[:], wosb[:, k * MOD:(k + 1) * MOD], cur[k][:],
                start=(k == 0), stop=(k == 3),
            )
        osb = const.tile([MOD, 128], fp32)
        nc.scalar.activation(osb[:], po[:], AF.Identity, bias=bosb[:MOD, :])
        nc.sync.dma_start(d["OUT"], osb[:])


def _host_prep(inputs):
    x = np.asarray(inputs["x"]).astype(np.int64)          # [B, T]
    emb = np.asarray(inputs["emb"], np.float32)           # [97, 512]
    Wf = np.asarray(inputs["Wf"], np.float32)
    bf = np.asarray(inputs["bf"], np.float32)
    Wb = np.asarray(inputs["Wb"], np.float32)
    bb = np.asarray(inputs["bb"], np.float32)
    W1 = np.asarray(inputs["W1"], np.float32)             # [512, 32768]
    b1 = np.asarray(inputs["b1"], np.float32)
    W2 = np.asarray(inputs["W2"], np.float32)
    b2 = np.asarray(inputs["b2"], np.float32)
    Wo = np.asarray(inputs["Wo"], np.float32)             # [97, 512]
    bo = np.asarray(inputs["bo"], np.float32)

    # fold embedding gather + input projection (exact same fp32 math:
    # (emb @ W.T)[x] == emb[x] @ W.T)
    Wfe = Wf @ emb.T + bf[:, None]                        # [512, 97]
    Wbe = Wb @ emb.T + bb[:, None]
    af = Wfe[:, x]                                        # [512, B, T], pos s = t
    ab = Wbe[:, x[:, ::-1]]                               # [512, B, T], generation order

    # scan layout: A[c, j=d*4+m, p, b*33+s], separator col s=32 stays -1e30
    A = np.full((NCORES, 8, 128, BL, CL), NEG, np.float32)
    for dd, src in ((0, af), (1, ab)):
        for m in range(4):
            sl = src[m * 128:(m + 1) * 128]               # [128, B, T]
            A[:, dd * 4 + m, :, :, :T] = sl.reshape(128, NCORES, BL, T).transpose(1, 0, 2, 3)
    A = A.reshape(NCORES, 8, 128, FREE).astype(BF16)

    # W1 -> [64, 128, 2048]: group g holds k-chunks 4g..4g+3 side by side
    W1S = np.ascontiguousarray(
        W1.T.reshape(W1_GRP, 4, 128, 512).transpose(0, 2, 1, 3).reshape(W1_GRP, 128, 2048)
    ).astype(BF16)
    W2S = np.ascontiguousarray(W2.T.reshape(4, 128, 512)).astype(BF16)
    WOS = np.ascontiguousarray(Wo.T.reshape(4, 128, MOD)).astype(BF16)
    B1 = b1.reshape(1, 512).astype(BF16)
    B2T = np.ascontiguousarray(b2.reshape(4, 128).T).astype(np.float32)
    BO = np.zeros((128, 1), np.float32)
    BO[:MOD, 0] = bo
    IDN = np.eye(128, dtype=np.float32).astype(BF16)

    shared = {"W1S": W1S, "W2S": W2S, "WOS": WOS, "B1": B1,
              "B2T": B2T, "BO": BO, "IDN": IDN}
    in_maps = [dict(shared, A=np.ascontiguousarray(A[c])) for c in range(NCORES)]
    return in_maps


def _get_nc():
    if "nc" not in _CACHE:
        _CACHE["nc"] = _build()
    return _CACHE["nc"]


def kernel(**inputs):
    from concourse.bass_utils import run_bass_kernel_spmd

    nc = _get_nc()
    in_maps = _host_prep(inputs)
    res = run_bass_kernel_spmd(nc, in_maps, list(range(NCORES)))
    outs = [np.asarray(res.results[c]["OUT"], np.float32) for c in range(NCORES)]
    return np.concatenate([o.T for o in outs], axis=0)   # [1024, 97]


# revision 25
# speedup vs baseline: 695.5206x; 695.5206x over previous
"""Trainium2 Bass kernel for the BiDirectionalRNN problem.

Math (matches the fp32 jax reference):
    e = emb[x]                                   # [B, T, 512]
    fwd:  h_t = relu(e_t @ Wf.T + bf + h_{t-1})  # fs[t]
    bwd over reversed e: bs[s]                   # generation order
    xcat = concat_t [fs[t], bs[t]]  -> [B, T*1024]
    h1 = relu(xcat @ W1.T + b1); 4x h = relu(h @ W2.T + b2); out = h @ Wo.T + bo

Strategy:
  * Data-parallel over batch: 1024/8 = 128 samples per NeuronCore.
  * Host folds embedding + input projection weights into per-direction
    tables WfeB = Wf @ emb.T + bf (512 x 97); the device computes the
    per-step drive terms a = WfeB @ onehot(x) with tiny K=97 matmuls from
    a shipped one-hot (0.8MB/dir vs 4.3MB of raw drive terms).
  * ScalarE copies each a-GEMM PSUM block into the scan layout
    [p, b*33 + s] (strided 3D AP), separator column = -1e30.
  * The whole 32-step recurrence h = relu(a + h_prev) runs as ONE DVE
    tensor_tensor_scan per (dir, hid-tile): state=(a add state) max 0,
    fp32 internal state; the separator resets state to 0 between chains.
  * W1 (64MB fp32) ships as bf16 (32MB/core) in 64 [128,2048] tiles,
    ordered (dir, m)-major so the GEMM starts after the first scan. The
    [B,32768]@[32768,512] GEMM accumulates all 256 k-chunks into a single
    PSUM bank; lhsT = strided views of the scan outputs.
  * Tail: PE-transpose h1, then 4 x [512,512] + [97,512] in transposed
    (feature-major) layout; all biases enter PSUM via rank-1 matmuls so
    each layer needs a single ScalarE activation.
"""

import numpy as np
import ml_dtypes

BF16 = ml_dtypes.bfloat16

MOD = 97
HID = 512
T = 32
B = 1024
NCORES = 8
BL = B // NCORES          # 128 batch per core
CL = T + 1                # chain length incl. separator column
FREE = BL * CL            # 4224 scan columns per tile
NEG = -1e30
W1_GRP = 64               # W1 DMA groups of 4 k-chunks (512KB each)

_CACHE: dict = {}


def _build():
    import concourse.tile as tile
    from concourse import bacc, mybir

    fp32 = mybir.dt.float32
    bf16 = mybir.dt.bfloat16

    nc = bacc.Bacc(
        "TRN2", target_bir_lowering=False, debug=False, num_devices=NCORES
    )

    d = {
        "IDA": nc.dram_tensor("IDA", [128, 129], bf16, kind="ExternalInput").ap(),
        "WFE": nc.dram_tensor("WFE", [MOD, 2 * HID], bf16, kind="ExternalInput").ap(),
        "W1S": nc.dram_tensor("W1S", [W1_GRP, 128, 2048], bf16, kind="ExternalInput").ap(),
        "W2O": nc.dram_tensor("W2O", [128, 4 * 512 + 4 * MOD], bf16, kind="ExternalInput").ap(),
        "BIA": nc.dram_tensor("BIA", [1, 1121 + 2 * BL * T], bf16, kind="ExternalInput").ap(),
        "OUT": nc.dram_tensor("OUT", [MOD, BL], fp32, kind="ExternalOutput").ap(),
    }

    with tile.TileContext(nc) as tc:
        _emit(tc, d, mybir)

    nc.compile()
    return nc


def _emit(tc, d, mybir):
    nc = tc.nc
    fp32 = mybir.dt.float32
    bf16 = mybir.dt.bfloat16
    AF = mybir.ActivationFunctionType
    ALU = mybir.AluOpType

    from contextlib import ExitStack

    with ExitStack() as ctx:
        const = ctx.enter_context(tc.tile_pool(name="const", bufs=1))
        a_pool = ctx.enter_context(tc.tile_pool(name="apool", bufs=2))
        h_pool = ctx.enter_context(tc.tile_pool(name="hpool", bufs=3))
        w1_pool = ctx.enter_context(tc.tile_pool(name="w1pool", bufs=24))
        hp_pool = ctx.enter_context(tc.tile_pool(name="hppool", bufs=3))
        ps_a = ctx.enter_context(tc.tile_pool(name="psa", bufs=2, space="PSUM"))
        ps_h1 = ctx.enter_context(tc.tile_pool(name="psh1", bufs=1, space="PSUM"))
        ps_t = ctx.enter_context(tc.tile_pool(name="pst", bufs=1, space="PSUM"))
        ps_l = ctx.enter_context(tc.tile_pool(name="psl", bufs=1, space="PSUM"))
        ps_o = ctx.enter_context(tc.tile_pool(name="pso", bufs=1, space="PSUM"))

        # ---- constants (merged DMAs to avoid early DMA-engine bubbles) ----
        wfe = const.tile([MOD, 2 * HID], bf16)
        nc.sync.dma_start(wfe[:], d["WFE"][:])
        w2o = const.tile([128, 4 * 512 + 4 * MOD], bf16)
        nc.sync.dma_start(w2o[:], d["W2O"][:])
        w2sb = w2o[:, 0:2048]
        wosb = w2o[:, 2048:2048 + 4 * MOD]
        bia = const.tile([1, 1121 + 2 * BL * T], bf16)
        nc.sync.dma_start(bia[:], d["BIA"])
        b1sb = bia[:, 0:512]
        b2r = bia[:, 512:1024]
        bor = bia[:, 1024:1121]
        xr = bia[:, 1121:1121 + 2 * BL * T]
        ida = const.tile([128, 129], bf16)
        nc.sync.dma_start(ida[:], d["IDA"])
        idsb = ida[:, 0:128]
        arn = ida[:, 128:129]
        ones = const.tile([1, 128], bf16)
        nc.vector.memset(ones[:], 1.0)
        zero = const.tile([128, 1], bf16)
        nc.vector.memset(zero[:], 0.0)
        # one-hot of x, built on device: replicate the x row over 97
        # partitions with a rank-1 matmul, then compare against arange
        ohall = const.tile([MOD, 2 * BL * T], bf16)
        ohsb = [ohall[:, 0:BL * T], ohall[:, BL * T:2 * BL * T]]

        # ---- drive terms + scans + linear1, interleaved per j = dir*4 + m ----
        # a = WfeB @ onehot in 8 PSUM blocks of 16 chains; ScalarE lays each
        # block into the scan layout [p, b*33 + s]; the DVE scan computes
        # h = relu(a + h_prev) for all 128 chains in one instruction; then
        # the two W1 groups for this j stream in and accumulate into psum_h1.
        # W1 group order is (dir, m)-major so group G only needs scan j = G//2.
        psum_h1 = ps_h1.tile([128, 512], fp32)

        def a_phase(j):
            dd, m = j // 4, j % 4
            a_sb = a_pool.tile([128, FREE], bf16, tag="a")
            sep = a_sb[:].rearrange("p (b t) -> p b t", t=CL)[:, :, T]
            nc.vector.memset(sep, NEG)
            lhsT = wfe[:, dd * HID + m * 128: dd * HID + m * 128 + 128]
            for q in range(8):
                if m == 0:
                    px = ps_a.tile([128, 512], fp32, tag="pa")
                    nc.tensor.matmul(
                        px[:MOD, :], ones[:, 0:MOD],
                        xr[:, dd * BL * T + q * 512: dd * BL * T + (q + 1) * 512],
                        start=True, stop=True,
                    )
                    nc.vector.tensor_tensor(
                        ohsb[dd][:, q * 512:(q + 1) * 512], px[:MOD, :],
                        arn[:MOD, :].broadcast_to([MOD, 512]),
                        op=mybir.AluOpType.is_equal,
                    )
                pa = ps_a.tile([128, 512], fp32, tag="pa")
                nc.tensor.matmul(
                    pa[:], lhsT, ohsb[dd][:, q * 512:(q + 1) * 512],
                    start=True, stop=True,
                )
                av = a_sb[:].rearrange("p (b t) -> p b t", t=CL)[:, 16 * q:16 * q + 16, 0:T]
                nc.scalar.copy(av, pa[:].rearrange("p (b t) -> p b t", t=T))
            h_t = h_pool.tile([128, FREE], bf16, tag="h")
            nc.vector.tensor_tensor_scan(
                h_t[:], a_sb[:], zero[:].broadcast_to([128, FREE]),
                initial=0.0, op0=ALU.add, op1=ALU.max,
            )
            return h_t

        hs = {0: a_phase(0), 1: a_phase(1)}
        for j in range(8):
            hv = hs[j][:].rearrange("p (b t) -> p t b", t=CL)
            for G in range(8 * j, 8 * j + 8):
                w_t = w1_pool.tile([128, 2048], bf16)
                nc.sync.dma_start(w_t[:], d["W1S"][G])
                for c in range(4):
                    t_idx = (G % 8) * 4 + c
                    nc.tensor.matmul(
                        psum_h1[:], hv[:, t_idx, :], w_t[:, c * 512:(c + 1) * 512],
                        start=(G == 0 and c == 0), stop=False,
                    )
                if G == 8 * j and j + 2 < 8:
                    hs[j + 2] = a_phase(j + 2)
        # rank-1 bias: ones.T @ b1 broadcasts b1 over the batch partitions
        nc.tensor.matmul(psum_h1[:], ones[:], b1sb, start=False, stop=True)

        # Tail latency trick: every PSUM->SBUF activation is split in column
        # halves so downstream matmuls depending only on the first half can
        # start while the second half is still on ScalarE.
        h1sb = const.tile([128, 512], bf16)
        nc.scalar.activation(h1sb[:], psum_h1[:], AF.Relu)

        # ---- transpose h1 to feature-major [512, 128] ----
        # Twin PSUM banks per stage: ScalarE drains one while VectorE drains
        # the other (Tile serializes same-bank readers, so one bank can't be
        # split across engines).
        pt_a = ps_t.tile([128, 256], bf16, tag="pta")
        pt_b = ps_t.tile([128, 256], bf16, tag="ptb")
        cur = hp_pool.tile([128, 512], bf16, tag="hp")
        for m in (0, 1):
            nc.tensor.transpose(
                pt_a[:, (m % 2) * 128:(m % 2) * 128 + 128],
                h1sb[:, m * 128:(m + 1) * 128], idsb[:])
        nc.scalar.copy(cur[:, 0:256], pt_a[:])
        for m in (2, 3):
            nc.tensor.transpose(
                pt_b[:, (m % 2) * 128:(m % 2) * 128 + 128],
                h1sb[:, m * 128:(m + 1) * 128], idsb[:])
        nc.vector.tensor_copy(cur[:, 256:512], pt_b[:])

        # ---- 4 x (h = relu(W2 @ h' + b2)), feature-major, col block = m ----
        for _L in range(4):
            pl_a = ps_l.tile([128, 256], fp32, tag="pla")
            pl_b = ps_l.tile([128, 256], fp32, tag="plb")
            for m in range(4):
                pl = pl_a if m < 2 else pl_b
                col = (m % 2) * 128
                for k in range(4):
                    nc.tensor.matmul(
                        pl[:, col:col + 128],
                        w2sb[:, k * 512 + m * 128: k * 512 + m * 128 + 128],
                        cur[:, k * 128:(k + 1) * 128],
                        start=(k == 0), stop=False,
                    )
                nc.tensor.matmul(
                    pl[:, col:col + 128],
                    b2r[:, m * 128:(m + 1) * 128], ones[:],
                    start=False, stop=True,
                )
            hq = hp_pool.tile([128, 512], bf16, tag="hp")
            nc.scalar.activation(hq[:, 0:256], pl_a[:], AF.Relu)
            nc.vector.tensor_scalar_max(hq[:, 256:512], pl_b[:], 0.0)
            cur = hq

        # ---- output head: out' = Wo @ h' + bo  -> [97, 128] ----
        po = ps_o.tile([MOD, 128], fp32, tag="po")
        for k in range(4):
            nc.tensor.matmul(
                po[:], wosb[:, k * MOD:(k + 1) * MOD], cur[:, k * 128:(k + 1) * 128],
                start=(k == 0), stop=False,
            )
        nc.tensor.matmul(po[:], bor, ones[:], start=False, stop=True)
        osb = const.tile([MOD, BL], fp32)
        nc.scalar.copy(osb[:], po[:])
        nc.sync.dma_start(d["OUT"], osb[:])


def _host_prep(inputs):
    x = np.asarray(inputs["x"]).astype(np.int64)          # [B, T]
    emb = np.asarray(inputs["emb"], np.float32)           # [97, 512]
    Wf = np.asarray(inputs["Wf"], np.float32)
    bf = np.asarray(inputs["bf"], np.float32)
    Wb = np.asarray(inputs["Wb"], np.float32)
    bb = np.asarray(inputs["bb"], np.float32)
    W1 = np.asarray(inputs["W1"], np.float32)             # [512, 32768]
    b1 = np.asarray(inputs["b1"], np.float32)
    W2 = np.asarray(inputs["W2"], np.float32)
    b2 = np.asarray(inputs["b2"], np.float32)
    Wo = np.asarray(inputs["Wo"], np.float32)             # [97, 512]
    bo = np.asarray(inputs["bo"], np.float32)

    # fold embedding gather + input projection + bias:
    # a_d[:, b, s] = (Wd @ emb.T + bd)[:, idx] since onehot has exactly one 1
    WFE = np.ascontiguousarray(np.stack([
        (Wf @ emb.T + bf[:, None]).T,                     # [97, 512]
        (Wb @ emb.T + bb[:, None]).T,
    ]).transpose(1, 0, 2).reshape(MOD, 2 * HID)).astype(BF16)

    # per-core x rows, col = b*32 + s; fwd s = t, bwd s = reversed t; the
    # device replicates these over 97 partitions and compares with arange
    # to build the one-hot (values 0..96 are exact in bf16)
    xc = x.reshape(NCORES, BL, T)
    XR = np.concatenate([
        xc.reshape(NCORES, BL * T), xc[:, :, ::-1].reshape(NCORES, BL * T)
    ], axis=1).astype(BF16)                               # [NC, 8192]
    IDA = np.concatenate([
        np.eye(128, dtype=np.float32),
        np.arange(128, dtype=np.float32).reshape(128, 1),
    ], axis=1).astype(BF16)

    # W1 -> [64, 128, 2048]: group G = (d, m, tg) holds k-chunks for
    # t = 4*tg .. 4*tg+3 of direction d, hid-tile m, side by side
    # W1.T row layout is [t, d, m, p]-major (xcat col = t*1024 + d*512 + m*128)
    W1S = np.ascontiguousarray(
        W1.T.reshape(8, 4, 2, 4, 128, 512)       # [tg, tc, d, m, p, col]
        .transpose(2, 3, 0, 4, 1, 5)             # [d, m, tg, p, tc, col]
        .reshape(W1_GRP, 128, 2048)
    ).astype(BF16)
    W2S = np.ascontiguousarray(W2.T.reshape(4, 128, 512).transpose(1, 0, 2).reshape(128, 2048)).astype(BF16)
    WOS = np.ascontiguousarray(Wo.T.reshape(4, 128, MOD).transpose(1, 0, 2).reshape(128, 4 * MOD)).astype(BF16)
    W2O = np.concatenate([W2S, WOS], axis=1)
    BIAH = np.concatenate([b1, b2, bo]).astype(BF16)      # [1121]

    shared = {"WFE": WFE, "W1S": W1S, "W2O": W2O, "IDA": IDA}
    in_maps = [
        dict(shared, BIA=np.concatenate([BIAH, XR[c]]).reshape(1, -1))
        for c in range(NCORES)
    ]
    return in_maps


def _get_nc():
    if "nc" not in _CACHE:
        _CACHE["nc"] = _build()
    return _CACHE["nc"]


def kernel(**inputs):
    from concourse.bass_utils import run_bass_kernel_spmd

    nc = _get_nc()
    in_maps = _host_prep(inputs)
    res = run_bass_kernel_spmd(nc, in_maps, list(range(NCORES)))
    outs = [np.asarray(res.results[c]["OUT"], np.float32) for c in range(NCORES)]
    return np.concatenate([o.T for o in outs], axis=0)   # [1024, 97]
